# revision 1
# baseline (speedup 1.0000x reference)
"""AttentionPooling Trainium2 kernel (8-core data-parallel SPMD), v2.

Reference computation per batch b (B=2048, T=200, E=H=64):
    att_in = [q, k, q-k, q*k]            (T, 4E)
    h   = elu(att_in @ W1 + b1)          (T, H)
    s   = h @ W2 + b2                    (T,)
    s   = where(mask, s, PAD); p = softmax(s)
    out = p @ k                          (E,)

Algebraic restructuring (v2):
  att_in @ W1 = c(q) + k @ Wk + (q*k) @ Wp     [c is a per-batch row]
  elu(x)+1 = max(x,0) + min(exp(x),1) exactly, so with m0 := min(z+c, 0):
    s ~ wk2.k + wp2.qk + w2.exp(m0) - w2.m0   (+ per-batch consts that
  softmax ignores), where wk2 = Wk@w2, wp2 = Wp@w2.  The relu part of the
  elu thus rides two LINEAR "score" matmuls directly on k and q*k, and the
  only element-wise ops per tile are one DVE min (m0) and one ACT exp.

Structure vs v1:
  * scores are computed TRANSPOSED: stationaries = k/qk/xm/m0 chunks
    [128, <=100], movings = tiny block-diag vectors [128, 2] -> sT [t, 2]
    in 2-cycle matmuls (replaces 16x200-cycle w2rep matmuls per group).
  * exp(sT) directly produces the weighted-sum moving operand eT [100, 64].
  * mask is baked into the weighted-sum keys on host (kn65 = [m*k | m]):
    per-batch stationary matmuls accumulate numerator AND denominator into
    fin65 [65, 32]; no mask tensor or mask multiply on-chip.
  * z PSUM tiles are [128, 1024] with duo slices at 512-col offsets so every
    matmul output stays inside one 2KB PSUM bank.
  * DMA packing: per group only kT + kn65 + out; all constants ship in one
    byte-packed image -> ~28 DMAs total (v1: 59).
  * element-wise assignment: ACT = exp + a half qk lane, DVE = m0 + one
    qk lane + fin copy, Pool = 2.5 qk lanes; the softmax division runs on
    host from the shipped [65, 32] numerator/denominator tiles.
"""

import os
import sys

import numpy as np

sys.path.insert(0, "/opt/trn_rl_repo")

import ml_dtypes

B, T, E, H = 2048, 200, 64, 64
NCORES = 8
BC = B // NCORES  # 256 batches per core
NPG = 16          # pairs per group
GB = 2 * NPG      # 32 batches per group
G = BC // GB      # 8 groups per core
TC = 100          # wsum t-chunk size (2 chunks of 100)

BF16 = ml_dtypes.bfloat16

_PROGRAM_CACHE = {}


def _build_program():
    import concourse.bass as bass
    import concourse.tile as tile
    from concourse import bacc, mybir

    f32 = mybir.dt.float32
    bf16 = mybir.dt.bfloat16
    OP = mybir.AluOpType
    AF = mybir.ActivationFunctionType

    nc = bacc.Bacc("TRN2", debug=False)

    kT_d = nc.dram_tensor("kT", [G, 128, NPG * T], bf16, kind="ExternalInput")
    kn65_d = nc.dram_tensor("kn65", [G, TC, 2 * GB * 65], bf16, kind="ExternalInput")
    crow_d = nc.dram_tensor("crow", [2, G * (NPG // 2) * 128], bf16, kind="ExternalInput")
    # one byte-packed const image per core:
    # [cbf bf16 [128,664] | qp f32 [128,128] | id65 f32 [65,65] row-padded]
    u8 = mybir.dt.uint8
    consts_d = nc.dram_tensor("consts", [128, 2100], u8, kind="ExternalInput")
    # unnormalized per-group result: rows 0:64 = sum(m*exp(s)*k) over t,
    # row 64 = sum(m*exp(s)); the division happens on host
    out_d = nc.dram_tensor("outp", [G, 65, GB], f32, kind="ExternalOutput")

    with tile.TileContext(nc) as tc:
        with (
            tc.tile_pool(name="const", bufs=1) as cp,
            tc.tile_pool(name="gload", bufs=3) as gp,
            tc.tile_pool(name="qk", bufs=8) as qkp,
            tc.tile_pool(name="acts", bufs=6) as ap_,
            tc.tile_pool(name="sm", bufs=4) as smp,
            tc.tile_pool(name="zps", bufs=3, space=bass.MemorySpace.PSUM) as zp,
            tc.tile_pool(name="sps", bufs=2, space=bass.MemorySpace.PSUM) as sp,
        ):
            consts = cp.tile([128, 2100], u8)
            nc.sync.dma_start(consts[:], consts_d[:])
            cbf = consts[:, 0:1328].bitcast(bf16)
            bdwk = cbf[:, 0:128]
            bdwp = cbf[:, 128:256]
            w2bd = cbf[:, 256:258]
            wk2bd = cbf[:, 258:260]
            wp2bd = cbf[:, 260:262]
            w2nbd = cbf[:, 262:264]
            ones_r = cbf[0:2, 264:264 + 2 * T]
            qpt = consts[:, 1328:1840].bitcast(f32)
            id65 = consts[0:65, 1840:2100].bitcast(f32)
            crow = cp.tile([2, G * (NPG // 2) * 128], bf16)
            nc.sync.dma_start(crow[:], crow_d[:])

            gstate = {}

            def emit_dma_head(g):
                kTg = gp.tile([128, NPG * T], bf16, tag="kTg")
                if g == 0:
                    nc.sync.dma_start(kTg[:, 0:2 * T], kT_d[g][:, 0:2 * T])
                    nc.sync.dma_start(kTg[:, 2 * T:8 * T], kT_d[g][:, 2 * T:8 * T])
                    nc.sync.dma_start(kTg[:, 8 * T:], kT_d[g][:, 8 * T:])
                else:
                    nc.sync.dma_start(kTg[:], kT_d[g])
                gstate[g] = dict(kTg=kTg)

            def emit_dma_tail(g):
                kn65g = gp.tile([TC, 2 * GB * 65], bf16, tag="kn65g")
                nc.sync.dma_start(kn65g[:], kn65_d[g])
                gstate[g].update(kn65g=kn65g)

            def emit_blk_qk(g, jj):
                # qk for one block = 2 duos (prefetched ahead of the z mms);
                # spread across Pool/DVE/ACT; tiles live on into the smm stage
                st = gstate[g]
                for d2 in range(2):
                    j0 = 4 * jj + 2 * d2
                    qk = qkp.tile([128, 2 * T], bf16, tag="qk")
                    for h in range(2):
                        src = st["kTg"][:, (j0 + h) * T:(j0 + h + 1) * T]
                        qcol = qpt[:, g * NPG + j0 + h:g * NPG + j0 + h + 1]
                        dst = qk[:, h * T:(h + 1) * T]
                        lane = 2 * d2 + h
                        if lane < 2 or (lane == 3 and jj % 2 == 0):
                            nc.gpsimd.tensor_scalar_mul(dst, src, qcol)
                        elif lane == 2:
                            nc.vector.tensor_scalar_mul(dst, src, qcol)
                        else:
                            nc.scalar.mul(dst, src, qcol)
                    st[("qk", j0 // 2)] = qk

            def emit_blk_zmm(g, jj):
                # one block = 4 pairs = 2 duos; z in one [128, 4T] PSUM tile
                st = gstate[g]
                # [128, 1024]: duo slices at 512-col offsets so each matmul
                # output stays inside one 2KB PSUM bank
                zsup = zp.tile([128, 1024], f32, tag="z")
                for d2 in range(2):
                    j0 = 4 * jj + 2 * d2
                    ksl = st["kTg"][:, j0 * T:(j0 + 2) * T]        # [128, 400]
                    qk = st[("qk", j0 // 2)]
                    zsl = zsup[:, 512 * d2: 512 * d2 + 2 * T]
                    nc.tensor.matmul(zsl, bdwk[:], ksl, start=True, stop=False)
                    nc.tensor.matmul(zsl, bdwp[:], qk[:], start=False, stop=False)
                    nc.tensor.matmul(
                        zsl, crow[0:2, (g * 8 + j0 // 2) * 128:(g * 8 + j0 // 2 + 1) * 128],
                        ones_r[:], start=False, stop=True,
                    )
                # s = wk2.k + wp2.qk + w2.xm - w2.m0 with m0 = min(z+c, 0),
                # xm = exp(m0); elu's relu-part rides the linear sT matmuls
                zv = zsup[:].rearrange("p (h c) -> p h c", h=2)[:, :, 0:2 * T]
                m0 = ap_.tile([128, 4 * T], bf16, tag="m0")
                m0v = m0[:].rearrange("p (h c) -> p h c", h=2)
                nc.vector.tensor_scalar_min(m0v, zv, 0.0)
                xm = ap_.tile([128, 4 * T], bf16, tag="xm")
                nc.scalar.activation(xm[:], m0[:], AF.Exp)
                st[("blk", jj)] = (m0, xm, 0)

            def emit_blk_smm(g, jj):
                # transposed score matmuls: stationary = u-chunk [128, 100],
                # moving = w2bd [128, 2] -> sT chunk [100, 2]
                st = gstate[g]
                m0, xm, ubase = st.pop(("blk", jj))
                if "tailp" not in st:
                    # [128, 164] PSUM: sT [0:100, 0:64], fin65 [0:65, 64:96],
                    # finT [0:32, 96:161]
                    tailp = sp.tile([128, 164], f32, tag="tailp")
                    st["tailp"] = tailp
                sT = st["tailp"][0:TC, 0:64]
                for j4 in range(4):
                    j = 4 * jj + j4
                    qk = st[("qk", j // 2)]
                    for c in range(2):
                        out = sT[:, c * GB + 2 * j:c * GB + 2 * j + 2]
                        usl = slice(ubase + j4 * T + c * TC,
                                    ubase + j4 * T + (c + 1) * TC)
                        ksl = slice(j * T + c * TC, j * T + (c + 1) * TC)
                        qsl = slice((j % 2) * T + c * TC, (j % 2) * T + (c + 1) * TC)
                        nc.tensor.matmul(
                            out, st["kTg"][:, ksl], wk2bd[:],
                            start=True, stop=False, skip_group_check=True)
                        nc.tensor.matmul(
                            out, qk[:, qsl], wp2bd[:],
                            start=False, stop=False, skip_group_check=True)
                        nc.tensor.matmul(
                            out, xm[:, usl], w2bd[:],
                            start=False, stop=False, skip_group_check=True)
                        nc.tensor.matmul(
                            out, m0[:, usl], w2nbd[:],
                            start=False, stop=True, skip_group_check=True)
                    if j % 2 == 1:
                        st.pop(("qk", j // 2))

            def emit_tail_sm(g):
                st = gstate[g]
                sT = st["tailp"][0:TC, 0:64]
                eT = smp.tile([TC, 64], bf16, tag="eT")
                nc.scalar.activation(eT[:], sT, AF.Exp)
                st["eT"] = eT

            def emit_tail_pe(g):
                st = gstate.pop(g)
                tailp = st["tailp"]
                eT = st["eT"]
                kn65g = st["kn65g"]
                fin65 = tailp[0:65, 64:96]
                for b in range(GB):
                    for c in range(2):
                        nc.tensor.matmul(
                            fin65[:, b:b + 1],
                            kn65g[:, (c * GB + b) * 65:(c * GB + b + 1) * 65],
                            eT[:, c * GB + b:c * GB + b + 1],
                            start=(c == 0), stop=(c == 1),
                            skip_group_check=True,
                        )
                fsb = smp.tile([65, 32], f32, tag="fsb")
                nc.vector.tensor_copy(fsb[:], fin65)
                nc.sync.dma_start(out_d[g], fsb[:])

            # software pipeline over blocks (4 pairs each): qk prefetched one
            # block ahead, smm lagged one block, tails spill into next group
            emit_dma_head(0)
            emit_dma_head(1)
            emit_dma_tail(0)
            emit_blk_qk(0, 0)
            for g in range(G):
                for jj in range(4):
                    B0 = 4 * g + jj          # global block index
                    if B0 + 1 < 4 * G:
                        emit_blk_qk((B0 + 1) // 4, (B0 + 1) % 4)
                    emit_blk_zmm(g, jj)
                    if jj == 0:
                        if g > 0:
                            emit_blk_smm(g - 1, 3)
                            emit_tail_sm(g - 1)
                    elif jj == 1:
                        if g > 0:
                            emit_tail_pe(g - 1)
                        emit_blk_smm(g, 0)
                    elif jj == 2:
                        if g + 2 < G:
                            emit_dma_head(g + 2)
                        if g + 1 < G:
                            emit_dma_tail(g + 1)
                        emit_blk_smm(g, 1)
                    else:
                        emit_blk_smm(g, jj - 1)
            emit_blk_smm(G - 1, 3)
            emit_tail_sm(G - 1)
            emit_tail_pe(G - 1)

    nc.compile()
    return nc


def _pack_inputs(queries, keys, mask, W1, b1, W2, b2):
    """Host-side packing into per-core input maps."""
    queries = np.asarray(queries, dtype=np.float32)
    keys = np.asarray(keys, dtype=np.float32)
    mask = np.asarray(mask)
    W1 = np.asarray(W1, dtype=np.float32)
    b1 = np.asarray(b1, dtype=np.float32)
    W2 = np.asarray(W2, dtype=np.float32)

    Wq = W1[0:E] + W1[2 * E:3 * E]        # query block + diff block
    Wk = W1[E:2 * E] - W1[2 * E:3 * E]    # key block - diff block
    Wp = W1[3 * E:4 * E]                  # product block

    # per-batch bias row c = q @ Wq + b1   -> (B, H)
    cvals = queries[:, 0, :] @ Wq + b1[None, :]

    # keys reshaped [core, group, pair, pb, t, e]
    K6 = keys.reshape(NCORES, G, NPG, 2, T, E)
    kT = np.ascontiguousarray(K6.transpose(0, 1, 3, 5, 2, 4)).reshape(
        NCORES, G, 128, NPG * T).astype(BF16)

    # wsum keys with mask baked in + denominator column:
    # kn65[core, g, t, (chunk, batch, 65)] ; batch = 2*pair + pb
    M4 = mask.reshape(NCORES, G, GB, T).astype(np.float32)
    K5 = K6.reshape(NCORES, G, GB, T, E)
    kn = np.concatenate([K5 * M4[..., None], M4[..., None]], axis=-1)  # [.., GB, T, 65]
    kn65 = np.ascontiguousarray(
        kn.reshape(NCORES, G, GB, 2, TC, 65).transpose(0, 1, 4, 3, 2, 5)
    ).reshape(NCORES, G, TC, 2 * GB * 65).astype(BF16)

    Q5 = queries[:, 0, :].reshape(NCORES, G, NPG, 2, E)
    qp = np.ascontiguousarray(Q5.transpose(0, 3, 4, 1, 2)).reshape(
        NCORES, 128, G * NPG).astype(np.float32)

    # duo layout: row r of crow holds pair (2*jj2 + r)'s c-row at free
    # offset (g*8 + jj2)*128
    crow = np.ascontiguousarray(
        cvals.reshape(NCORES, G * (NPG // 2), 2, 128).transpose(0, 2, 1, 3)
    ).reshape(NCORES, 2, G * (NPG // 2) * 128).astype(BF16)

    bdwk = np.zeros((128, 128), np.float32)
    bdwk[0:64, 0:64] = Wk
    bdwk[64:128, 64:128] = Wk
    bdwp = np.zeros((128, 128), np.float32)
    bdwp[0:64, 0:64] = Wp
    bdwp[64:128, 64:128] = Wp

    def bdcol(v):
        m = np.zeros((128, 2), np.float32)
        m[0:64, 0] = v
        m[64:128, 1] = v
        return m

    w2bd = bdcol(W2[:, 0])
    wk2bd = bdcol(Wk @ W2[:, 0])
    wp2bd = bdcol(Wp @ W2[:, 0])
    w2nbd = bdcol(-W2[:, 0])

    onesr = np.zeros((128, 2 * T), np.float32)
    onesr[0, 0:T] = 1.0
    onesr[1, T:2 * T] = 1.0
    cbf = np.concatenate([bdwk, bdwp, w2bd, wk2bd, wp2bd, w2nbd, onesr], axis=1).astype(BF16)
    id65 = np.zeros((128, 65), np.float32)
    id65[0:65] = np.eye(65, dtype=np.float32)

    in_maps = []
    for c in range(NCORES):
        consts = np.concatenate([
            cbf.view(np.uint8).reshape(128, -1),
            qp[c].view(np.uint8).reshape(128, -1),
            id65.view(np.uint8).reshape(128, -1),
        ], axis=1)
        m = {"kT": kT[c], "kn65": kn65[c], "crow": crow[c], "consts": consts}
        in_maps.append(m)
    return in_maps


def kernel(queries, keys, mask, W1, b1, W2, b2):
    from concourse import bass_utils

    key = "prog"
    if key not in _PROGRAM_CACHE:
        _PROGRAM_CACHE[key] = _build_program()
    nc = _PROGRAM_CACHE[key]

    in_maps = _pack_inputs(queries, keys, mask, W1, b1, W2, b2)
    res = bass_utils.run_bass_kernel_spmd(nc, in_maps, list(range(NCORES)))
    outs = [res.results[c]["outp"] for c in range(NCORES)]  # [G, 65, GB] each
    fin = np.stack(outs).astype(np.float32)                 # [NC, G, 65, GB]
    out = (fin[:, :, 0:64, :] / fin[:, :, 64:65, :]).transpose(0, 1, 3, 2)
    return np.ascontiguousarray(out.reshape(B, E))[:, None, :]


if __name__ == "__main__":
    sys.path.insert(0, os.path.dirname(os.path.abspath(__file__)))
    import reference

    inputs = reference.setup_inputs()
    expected = np.asarray(reference.reference(**inputs))
    actual = kernel(**{k: np.asarray(v) for k, v in inputs.items()})
    err = np.abs(actual - expected).max()
    rel = err / max(1e-12, np.abs(expected).max())
    print("absmax err:", err, "rel:", rel)

